# revision 25
# baseline (speedup 1.0000x reference)
"""MultiOutSizeLinear (MoE routed linear), Trainium2 x8 — weight-stationary.

Host side: route tokens to experts by ``out_feat_size``; balance each
expert's tokens evenly across the 8 cores (shared capacities so one SPMD
program serves all cores); gather + transpose each core's tokens into
chunk-blocked x^T in bf16. Device: the matmul stream is weight-stationary
with 4 interleaved PSUM accumulation chains:

  for each expert block (up to 4 chains of <=512 tokens):
    for each 128-col tile ct of the expert:
      for kk in 8 K-tiles:             # lhsT = W^T col tile, constant
        for chain j:                   #   across the 4-MM run
          psum_j[128 cols, n_j tok] += wT[ct,kk].T @ xT[kk, chain_j]

Consecutive MMs always hit different PSUM banks (4 chains x double buffer =
8 banks), the stationary operand only changes once per 4 MMs, and every MM
is N<=512 moving tokens. Measured on this part this stream shape sustains
the best PE clock under the chip's all-cores-active throttle.

Outputs are column-major per expert ([cts, 128 cols, caps] bf16); the host
transposes back. Bias is added on eviction from a host-prebroadcast
[128, 15*512] f32 tile.
"""

import sys
import numpy as np

sys.path.insert(0, "/opt/trn_rl_repo")

OUT_SIZES = (128, 256, 512, 1024)
N_EXPERTS = len(OUT_SIZES)
IN_FEAT = 1024
N_CORES = 8
K_TILES = IN_FEAT // 128
CHUNK = 512
WOFF = tuple(int(np.cumsum((0,) + OUT_SIZES)[k]) for k in range(N_EXPERTS))
W_COLS = sum(OUT_SIZES)
CBLOCKS = W_COLS // 128  # 15

_nc_cache: dict = {}


def _plan_chains(caps, seg_start, seg_order):
    """Per-expert chain list [(chunk, off, n, tokrow)] and block partition."""
    tpad = sum(caps)

    def expert_of(tok):
        for k in seg_order:
            if tok < seg_start[k] + caps[k]:
                return k
        raise AssertionError

    chains = {k: [] for k in seg_order}
    for c in range(tpad // CHUNK):
        g = 0
        while g < CHUNK:
            tok = c * CHUNK + g
            k = expert_of(tok)
            end = min(seg_start[k] + caps[k] - c * CHUNK, CHUNK)
            chains[k].append((c, g, end - g, tok - seg_start[k]))
            g = end

    def sizes(m, lead2=False):
        # lead2: start with a 2-chain block so the first MMs begin after
        # only two chunk DMAs (faster pipeline fill at the loop boundary)
        out = []
        if lead2 and m >= 5:
            out.append(2)
            m -= 2
        while m > 0:
            if m == 5:
                out += [3, 2]
                m = 0
            elif m >= 4:
                out.append(4)
                m -= 4
            else:
                out.append(m)
                m = 0
        return out

    # flush[c] = list of (expert, [chains]) whose last chain is in chunk c
    flush = {}
    for k in seg_order:
        i = 0
        for s in sizes(len(chains[k]), lead2=(k == seg_order[0])):
            grp = chains[k][i:i + s]
            i += s
            flush.setdefault(grp[-1][0], []).append((k, grp))
    return flush


def _build(caps, repeat=1, loop=None, xbufs=12, obufs=8,
           staggered=True):
    import concourse.bacc as bacc
    import concourse.mybir as mybir
    import concourse.tile as tile

    f32 = mybir.dt.float32
    bf16 = mybir.dt.bfloat16
    tpad = sum(caps)
    assert tpad % CHUNK == 0

    nc = bacc.Bacc(None, target_bir_lowering=False, debug=False)
    xt = nc.dram_tensor("xt", [tpad // CHUNK, IN_FEAT, CHUNK], bf16,
                        kind="ExternalInput")
    wt = nc.dram_tensor("wt", [IN_FEAT, W_COLS], bf16, kind="ExternalInput")
    bb = nc.dram_tensor("bb", [128, CBLOCKS * 512], f32, kind="ExternalInput")
    outs = {}
    for k in range(N_EXPERTS):
        if caps[k]:
            cts = OUT_SIZES[k] // 128
            outs[k] = nc.dram_tensor(f"out{k}c", [cts, 128, caps[k]], bf16,
                                     kind="ExternalOutput")

    seg_order = [k for k in (3, 2, 1, 0) if caps[k] > 0]
    seg_start = {}
    t0 = 0
    for k in seg_order:
        seg_start[k] = t0
        t0 += caps[k]

    flush = _plan_chains(caps, seg_start, seg_order)

    with tile.TileContext(nc) as tc:
        with (
            tc.tile_pool(name="const", bufs=1) as const,
            tc.tile_pool(name="xp", bufs=xbufs) as xp,
            tc.tile_pool(name="op", bufs=obufs) as op,
            tc.tile_pool(name="psw", bufs=2, space="PSUM") as psw,
        ):
            wt_sb = const.tile([128, K_TILES, W_COLS], bf16)
            nc.sync.dma_start(wt_sb[:], wt.rearrange("(kk p) n -> p kk n", p=128))
            bb_sb = const.tile([128, CBLOCKS * 512], f32)
            nc.sync.dma_start(bb_sb[:], bb[:])

            def emit_block(k, grp, x_tiles):
                cts = OUT_SIZES[k] // 128
                for ct in range(cts):
                    cb = (WOFF[k] + ct * 128) // 128
                    pss = [psw.tile([128, 512], f32, tag=f"w{j}",
                                    name=f"psw{j}") for j in range(len(grp))]
                    for kk in range(K_TILES):
                        for j, (ci, off, n, row) in enumerate(grp):
                            nc.tensor.matmul(
                                pss[j][:, :n],
                                wt_sb[:, kk,
                                      WOFF[k] + ct * 128:WOFF[k] + ct * 128 + 128],
                                x_tiles[ci][:, kk, off:off + n],
                                start=(kk == 0), stop=(kk == K_TILES - 1))
                    for j, (ci, off, n, row) in enumerate(grp):
                        o_sb = op.tile([128, 512], bf16, tag="o", name="o_sb")
                        nc.vector.tensor_add(
                            o_sb[:, :n], pss[j][:, :n],
                            bb_sb[:, cb * 512:cb * 512 + n])
                        nc.sync.dma_start(outs[k][ct, :, row:row + n],
                                          o_sb[:, :n])

            def body():
                x_tiles = {}
                for c in range(tpad // CHUNK):
                    x_sb = xp.tile([128, K_TILES, CHUNK], bf16, tag="x",
                                   name="x_sb")
                    nc.scalar.dma_start(
                        x_sb[:],
                        xt[c].rearrange("(kk p) t -> p kk t", p=128))
                    x_tiles[c] = x_sb
                    for k, grp in flush.get(c, []):
                        emit_block(k, grp, x_tiles)

            if loop:
                with tc.For_i(0, loop, 1, staggered_reset=staggered):
                    for _ in range(repeat):
                        body()
            else:
                for _ in range(repeat):
                    body()
    nc.compile()
    return nc


def _get_nc(caps, repeat=1, loop=None):
    key = (tuple(caps), repeat, loop)
    if key not in _nc_cache:
        _nc_cache[key] = _build(caps, repeat=repeat, loop=loop)
    return _nc_cache[key]


def _route(out_feat_size):
    ofs = np.asarray(out_feat_size).astype(np.int64).reshape(-1)
    branch = np.full(ofs.shape, -1, dtype=np.int64)
    for k, s in enumerate(OUT_SIZES):
        branch[ofs == s] = k
    return branch


def _plan(branch):
    idx_all = {k: np.nonzero(branch == k)[0] for k in range(N_EXPERTS)}
    per_core = [int(-(-len(idx_all[k]) // N_CORES)) for k in range(N_EXPERTS)]
    caps = [int(-(-per_core[k] // 128) * 128) for k in range(N_EXPERTS)]
    rem = sum(caps) % 512
    if rem:
        for k in (0, 1, 2, 3):  # pad the cheapest non-empty expert
            if caps[k]:
                caps[k] += 512 - rem
                break
    return idx_all, tuple(caps)


def kernel(x, weight, bias, out_feat_size):
    import ml_dtypes
    from concourse.bass_utils import run_bass_kernel_spmd

    bf16 = np.dtype(ml_dtypes.bfloat16)
    x = np.asarray(x, dtype=np.float32)
    weight = np.asarray(weight, dtype=np.float32)
    bias = np.asarray(bias, dtype=np.float32)
    B, T, D = x.shape
    assert D == IN_FEAT
    n_tok = B * T

    branch = _route(out_feat_size)
    idx_all, caps = _plan(branch)
    if sum(caps) == 0:
        return np.zeros((B, T, IN_FEAT), dtype=np.float32)

    wt = np.empty((IN_FEAT, W_COLS), dtype=np.float32)
    bb = np.empty((W_COLS,), dtype=np.float32)
    for k, ok in enumerate(OUT_SIZES):
        wt[:, WOFF[k]:WOFF[k] + ok] = weight[k, :ok, :].T
        bb[WOFF[k]:WOFF[k] + ok] = bias[k, :ok]
    wt = wt.astype(bf16)
    # bias pre-broadcast: [128, cb*512 + t] = bb[cb*128 + p]
    bb_bc = np.ascontiguousarray(
        np.repeat(bb.reshape(CBLOCKS, 128).T[:, :, None], 512, axis=2)
        .reshape(128, CBLOCKS * 512))

    x2 = x.reshape(n_tok, IN_FEAT).astype(bf16)
    tpad = sum(caps)
    seg_off = {}
    t0 = 0
    for k in (3, 2, 1, 0):
        if caps[k]:
            seg_off[k] = t0
            t0 += caps[k]

    in_maps = []
    core_slices = []
    for c in range(N_CORES):
        perm = np.zeros(tpad, dtype=np.int64)
        slices = {}
        for k, off in seg_off.items():
            idx = idx_all[k]
            m = int(-(-len(idx) // N_CORES))
            part = idx[c * m:(c + 1) * m]
            slices[k] = part
            if len(part):
                perm[off:off + len(part)] = part
                perm[off + len(part):off + caps[k]] = part[0]
        xtb = np.empty((tpad // CHUNK, IN_FEAT, CHUNK), dtype=bf16)
        for ci in range(tpad // CHUNK):
            np.copyto(xtb[ci], x2[perm[ci * CHUNK:(ci + 1) * CHUNK]].T)
        in_maps.append({"xt": xtb, "wt": wt, "bb": bb_bc})
        core_slices.append(slices)

    global _LAST_CAPS, _LAST_IN_MAPS
    _LAST_CAPS, _LAST_IN_MAPS = caps, in_maps

    nc = _get_nc(caps)
    res = run_bass_kernel_spmd(nc, in_maps, list(range(N_CORES))).results

    out = np.zeros((n_tok, IN_FEAT), dtype=np.float32)
    for c in range(N_CORES):
        for k, part in core_slices[c].items():
            n = len(part)
            if n == 0:
                continue
            ok = OUT_SIZES[k]
            r = res[c][f"out{k}c"][:, :, :n].astype(np.float32)
            out[part, :ok] = np.transpose(r, (2, 0, 1)).reshape(n, ok)
            if ok < IN_FEAT:
                out[part, ok:] = bias[k, ok:]
    return out.reshape(B, T, IN_FEAT)
